# revision 1
# baseline (speedup 1.0000x reference)
"""BiGRU Trainium2 kernel (Bass/Tile), SPMD over 8 NeuronCores.

Sharding: data-parallel over batch (16 rows/core); each core runs BOTH GRU
directions (fwd + time-reversed bwd) as two independent dependency chains so
the Tile scheduler can overlap PE/ACT/DVE/GPSIMD across them.

Per-core, per-step layout (B=16, H=512):
  gates in [batch-part, H-free] layout; four PSUM tiles per direction
  (r, zneg, nh, nx), each in its own bank at a 32-aligned partition strip
  (0/32/64/96) so the matmuls go to distinct tensor-engine column groups
  and run concurrently.
  r-tile   = b_r  + x_t@Wih_r.T  + h@Whh_r.T        (7 MMs, N=512)
  zn-tile  = -(b_z + x_t@Wih_z.T + h@Whh_z.T)       (weights pre-negated on
             host so z' = 1-z = sigmoid(zn) directly)
  nh-tile  = b_hn + h@Whh_n.T                        (5 MMs)
  nx-tile  = b_in + x_t@Wih_n.T                      (3 MMs)
  r = sig(r-tile); z' = sig(zn-tile); n = tanh(r*nh + nx)
  h' = h + z'*(n - h)
  hT (lhsT layout, [128part, 4*16]) rebuilt via 4 PE transposes + 1 copy.

All matmul operands bf16 (fp32 PSUM accumulation); CPU simulation of this
exact rounding gives max rel err ~1e-4 vs the fp32 reference.
"""

import numpy as np
import ml_dtypes

import concourse.bass as bass
import concourse.bacc as bacc
import concourse.mybir as mybir
from concourse import tile
from concourse.bass_utils import run_bass_kernel_spmd

BF = ml_dtypes.bfloat16
V, E, H = 50000, 256, 512
B, T = 128, 512
NC = 8
BL = B // NC          # 16 batch rows per core
G = 3 * H             # 1536
EK = E // 128         # 2 contraction chunks for x
HK = H // 128         # 4 contraction chunks for h

bf = mybir.dt.bfloat16
f32 = mybir.dt.float32


def _build_nc():
    nc = bacc.Bacc(None, target_bir_lowering=False)

    xT_f = nc.dram_tensor("xT_f", [EK, 128, T * BL], bf, kind="ExternalInput")
    xT_b = nc.dram_tensor("xT_b", [EK, 128, T * BL], bf, kind="ExternalInput")
    WihT = {d: nc.dram_tensor(f"WihT_{d}", [EK, 128, G], bf, kind="ExternalInput")
            for d in "fb"}
    WhhT = {d: nc.dram_tensor(f"WhhT_{d}", [HK, 128, G], bf, kind="ExternalInput")
            for d in "fb"}
    bias = {d: nc.dram_tensor(f"bias_{d}", [1, 4 * H], bf, kind="ExternalInput")
            for d in "fb"}
    fcw = {d: nc.dram_tensor(f"fcw_{d}", [HK, 128, 1], bf, kind="ExternalInput")
           for d in "fb"}
    fcb = nc.dram_tensor("fcb", [BL, 1], f32, kind="ExternalInput")
    ones = nc.dram_tensor("ones", [1, BL], bf, kind="ExternalInput")
    ident = nc.dram_tensor("ident", [BL, BL], bf, kind="ExternalInput")
    out = nc.dram_tensor("out", [BL, 1], f32, kind="ExternalOutput")

    ACT = mybir.ActivationFunctionType
    with tile.TileContext(nc) as tc:
        with (
            tc.tile_pool(name="cst", bufs=1) as cst,
            tc.tile_pool(name="wk", bufs=3) as wk,
            tc.tile_pool(name="ps", bufs=1, space="PSUM") as ps,
        ):
            # ---- resident SBUF state ----
            xT_sb = {}
            for d, src in (("f", xT_f), ("b", xT_b)):
                t_ = cst.tile([128, EK * T * BL], bf, tag=f"xT{d}", name=f"xT{d}")
                for e in range(EK):
                    nc.sync.dma_start(t_[:, e * T * BL:(e + 1) * T * BL], src[e])
                xT_sb[d] = t_
            wih_sb, whh_sb, bias_sb, fcw_sb = {}, {}, {}, {}
            for d in "fb":
                w1 = cst.tile([128, EK * G], bf, tag=f"wih{d}", name=f"wih{d}")
                for e in range(EK):
                    nc.sync.dma_start(w1[:, e * G:(e + 1) * G], WihT[d][e])
                wih_sb[d] = w1
                w2 = cst.tile([128, HK * G], bf, tag=f"whh{d}", name=f"whh{d}")
                for k in range(HK):
                    nc.sync.dma_start(w2[:, k * G:(k + 1) * G], WhhT[d][k])
                whh_sb[d] = w2
                bz = cst.tile([1, 4 * H], bf, tag=f"bias{d}", name=f"bias{d}")
                nc.sync.dma_start(bz[:, :], bias[d][:, :])
                bias_sb[d] = bz
                fw = cst.tile([128, HK], bf, tag=f"fcw{d}", name=f"fcw{d}")
                for k in range(HK):
                    nc.sync.dma_start(fw[:, k:k + 1], fcw[d][k])
                fcw_sb[d] = fw
            fcb_sb = cst.tile([BL, 1], f32, tag="fcb")
            nc.sync.dma_start(fcb_sb[:, :], fcb[:, :])
            ones_sb = cst.tile([1, BL], bf, tag="ones")
            nc.sync.dma_start(ones_sb[:, :], ones[:, :])
            id_sb = cst.tile([BL, BL], bf, tag="ident")
            nc.sync.dma_start(id_sb[:, :], ident[:, :])

            # persistent state (h in both layouts), zero-initialized
            h_sb = {d: cst.tile([BL, H], bf, tag=f"h{d}", name=f"h{d}") for d in "fb"}
            hT_sb = {d: cst.tile([128, HK * BL], bf, tag=f"hT{d}", name=f"hT{d}") for d in "fb"}
            for d in "fb":
                nc.vector.memzero(h_sb[d][:, :])
                nc.vector.memzero(hT_sb[d][:, :])

            # absorb DMA-completion waits one-per-instruction (the PE
            # Ldweights microinstruction can carry only a single sync wait,
            # so no in-loop matmul may depend on >1 outstanding DMA/engine)
            warm_ps = ps.tile([128, H], f32, tag="g_rf", name="warm_ps")
            scrap = cst.tile([1, BL], bf, tag="scrap")
            first_w = True
            for src_ap in ([wih_sb[d][0:1, e * G:e * G + BL] for d in "fb" for e in range(EK)]
                           + [whh_sb[d][0:1, k * G:k * G + BL] for d in "fb" for k in range(HK)]
                           + [bias_sb[d][0:1, 0:BL] for d in "fb"]
                           + [fcw_sb[d][0:1, 0:HK] for d in "fb"]
                           + [ones_sb[0:1, 0:BL], id_sb[0:1, 0:BL]]):
                nc.tensor.matmul(warm_ps[0:1, 0:src_ap.free_size()],
                                 ones_sb[:, 0:1], src_ap,
                                 start=first_w, stop=False)
                first_w = False
            nc.tensor.matmul(warm_ps[0:1, 0:1], ones_sb[:, 0:1],
                             ones_sb[:, 0:1], start=False, stop=True)
            for d in "fb":
                for e in range(EK):
                    nc.vector.tensor_copy(scrap[0:1, :],
                                          xT_sb[d][0:1, e * T * BL:e * T * BL + BL])
            nc.scalar.activation(scrap[0:1, :], scrap[0:1, :],
                                 mybir.ActivationFunctionType.Sigmoid)

            # partition strip per gate: r@0, zn@32, nh@64, nx@0
            STRIP = {"r": 0, "zn": 32, "nh": 64, "nx": 96}
            # bias columns in bias_sb: r 0:512, zn 512:1024, nh 1024:1536, nx 1536:2048
            BCOL = {"r": 0, "zn": H, "nh": 2 * H, "nx": 3 * H}
            # gate column block in the weight tensors (r, z, n)
            WCOL = {"r": 0, "zn": H, "nh": 2 * H, "nx": 2 * H}

            def step_mm(tix, d):
                """Matmul phase of one GRU timestep for direction d."""
                # stage x_t at a static SBUF address (ldweights can't take
                # register offsets); GPSIMD so the DVE queue stays clear
                xcur = wk.tile([128, EK * BL], bf, tag=f"xcur{d}", name=f"xcur{d}")
                for e in range(EK):
                    nc.gpsimd.tensor_copy(
                        xcur[:, e * BL:(e + 1) * BL],
                        xT_sb[d][:, bass.ds(tix + e * T * BL, BL)])
                # one PSUM bank per gate (start=True clear is bank-wide and
                # races concurrent col-group writes if strips share a bank)
                g = {}
                for gname in ("r", "zn", "nh", "nx"):
                    g[gname] = ps.tile([128, H], f32, tag=f"g_{gname}{d}", name=f"g_{gname}{d}")
                for gname in ("r", "zn", "nh", "nx"):
                    s = STRIP[gname]
                    pos = (0, s)
                    o = g[gname][s:s + BL, :]
                    nc.tensor.matmul(
                        o, ones_sb[:, :], bias_sb[d][:, BCOL[gname]:BCOL[gname] + H],
                        start=True, stop=False, tile_position=pos)
                    wc = WCOL[gname]
                    if gname in ("r", "zn", "nx"):   # x-projection terms
                        for e in range(EK):
                            nc.tensor.matmul(
                                o, xcur[:, e * BL:(e + 1) * BL],
                                wih_sb[d][:, e * G + wc: e * G + wc + H],
                                start=False,
                                stop=(gname == "nx" and e == EK - 1),
                                tile_position=pos)
                    if gname in ("r", "zn", "nh"):   # h-projection terms
                        for k in range(HK):
                            nc.tensor.matmul(
                                o, hT_sb[d][:, k * BL:(k + 1) * BL],
                                whh_sb[d][:, k * G + wc: k * G + wc + H],
                                start=False, stop=(k == HK - 1),
                                tile_position=pos)
                return g

            def step_vec(g, d):
                """Gate math for direction d.
                h' = z'*n + z*h with the z*h branch computed off-chain."""
                r = wk.tile([BL, H], bf, tag=f"r{d}", name=f"r{d}")
                zp = wk.tile([BL, H], bf, tag=f"zp{d}", name=f"zp{d}")
                n = wk.tile([BL, H], bf, tag=f"n{d}", name=f"n{d}")
                v = wk.tile([BL, H], bf, tag=f"v{d}", name=f"v{d}")
                zf = wk.tile([BL, H], bf, tag=f"zf{d}", name=f"zf{d}")
                zh = wk.tile([BL, H], bf, tag=f"zh{d}", name=f"zh{d}")
                zn = wk.tile([BL, H], bf, tag=f"zn{d}", name=f"zn{d}")
                h = h_sb[d]
                nc.scalar.activation(r[:, :], g["r"][0:BL, :], ACT.Sigmoid)
                nc.scalar.activation(zp[:, :], g["zn"][32:32 + BL, :], ACT.Sigmoid)
                # off-chain z branch on GPSIMD: z = 1 - z', zh = z*h
                nc.gpsimd.tensor_scalar(zf[:, :], zp[:, :], -1.0, 1.0,
                                        mybir.AluOpType.mult, mybir.AluOpType.add)
                nc.gpsimd.tensor_mul(zh[:, :], zf[:, :], h[:, :])
                # n chain
                nc.vector.tensor_mul(v[:, :], r[:, :], g["nh"][64:64 + BL, :])
                nc.vector.tensor_add(v[:, :], v[:, :], g["nx"][96:96 + BL, :])
                nc.scalar.activation(n[:, :], v[:, :], ACT.Tanh)
                nc.vector.tensor_mul(zn[:, :], zp[:, :], n[:, :])
                nc.vector.tensor_add(h[:, :], zn[:, :], zh[:, :])

            def step_tr(d):
                """Rebuild transposed state for next step's lhsT."""
                h = h_sb[d]
                tr = ps.tile([128, HK * BL], bf, tag=f"g_nx{d}", name=f"tr{d}")
                for k in range(HK):
                    nc.tensor.matmul(
                        tr[:, k * BL:(k + 1) * BL],
                        h[:, k * 128:(k + 1) * 128], id_sb[:, :],
                        is_transpose=True, start=(k == 0), stop=(k == HK - 1))
                nc.scalar.copy(hT_sb[d][:, :], tr[:, :])

            U = 32
            with tc.For_i(0, T // U, 1, staggered_reset=True, hint_engines=(mybir.EngineType.PE,)) as it:
                for u in range(U):
                    tix = it * (U * BL) + u * BL
                    gf = step_mm(tix, "f")
                    gb = step_mm(tix, "b")
                    step_vec(gf, "f")
                    step_vec(gb, "b")
                    step_tr("f")
                    step_tr("b")

            # ---- final FC: sigmoid(h_f . wf + h_b . wb + b) ----
            fc_ps = ps.tile([BL, 1], f32, tag="g_nxf")
            first = True
            for d in "fb":
                for k in range(HK):
                    nc.tensor.matmul(
                        fc_ps[:, :], hT_sb[d][:, k * BL:(k + 1) * BL],
                        fcw_sb[d][:, k:k + 1],
                        start=first, stop=(d == "b" and k == HK - 1))
                    first = False
            o_sb = wk.tile([BL, 1], f32, tag="o")
            nc.scalar.activation(o_sb[:, :], fc_ps[:, :], ACT.Sigmoid,
                                 bias=fcb_sb[:, 0:1])
            nc.sync.dma_start(out[:, :], o_sb[:, :])
    nc.finalize()
    return nc


_NC_CACHE = None


def _get_nc():
    global _NC_CACHE
    if _NC_CACHE is None:
        _NC_CACHE = _build_nc()
    return _NC_CACHE


def _prep_core(x_c, rev):
    """x_c [BL, T, E] f32 -> [EK, 128, T*BL] bf16 (optionally time-reversed)."""
    if rev:
        x_c = x_c[:, ::-1, :]
    # xT[e, p, t*BL + b] = x_c[b, t, 128e + p]
    xt = np.ascontiguousarray(x_c.transpose(2, 1, 0)).reshape(EK, 128, T * BL)
    return xt.astype(BF)


def _prep_weights(W_ih, W_hh, b_ih, b_hh):
    Wi = np.array(W_ih, np.float32).copy()
    Wh = np.array(W_hh, np.float32).copy()
    Wi[H:2 * H] *= -1.0
    Wh[H:2 * H] *= -1.0
    wihT = np.ascontiguousarray(Wi.T).reshape(EK, 128, G).astype(BF)
    whhT = np.ascontiguousarray(Wh.T).reshape(HK, 128, G).astype(BF)
    bsum = np.asarray(b_ih, np.float32) + np.asarray(b_hh, np.float32)
    bias = np.concatenate([
        bsum[0:H], -bsum[H:2 * H],
        np.asarray(b_hh, np.float32)[2 * H:3 * H],
        np.asarray(b_ih, np.float32)[2 * H:3 * H]]).reshape(1, 4 * H).astype(BF)
    return wihT, whhT, bias


def prepare_in_maps(inputs, emb, W_ih_f, W_hh_f, b_ih_f, b_hh_f,
                    W_ih_b, W_hh_b, b_ih_b, b_hh_b, fc_w, fc_b):
    ids = np.asarray(inputs)
    emb = np.asarray(emb, np.float32)
    x = emb[ids]  # [B, T, E]

    wihT_f, whhT_f, bias_f = _prep_weights(W_ih_f, W_hh_f, b_ih_f, b_hh_f)
    wihT_b, whhT_b, bias_b = _prep_weights(W_ih_b, W_hh_b, b_ih_b, b_hh_b)
    fc = np.asarray(fc_w, np.float32)[0]
    fcw_f = fc[0:H].reshape(HK, 128, 1).astype(BF)
    fcw_b = fc[H:2 * H].reshape(HK, 128, 1).astype(BF)
    fcb = np.full((BL, 1), np.float32(np.asarray(fc_b).reshape(-1)[0]), np.float32)
    ones = np.ones((1, BL), BF)
    ident = np.eye(BL, dtype=BF)

    shared = dict(WihT_f=wihT_f, WihT_b=wihT_b, WhhT_f=whhT_f, WhhT_b=whhT_b,
                  bias_f=bias_f, bias_b=bias_b, fcw_f=fcw_f, fcw_b=fcw_b,
                  fcb=fcb, ones=ones, ident=ident)
    in_maps = []
    for c in range(NC):
        x_c = x[c * BL:(c + 1) * BL]
        in_maps.append(dict(shared,
                            xT_f=_prep_core(x_c, False),
                            xT_b=_prep_core(x_c, True)))
    return in_maps


def kernel(**inputs):
    in_maps = prepare_in_maps(**inputs)
    nc = _get_nc()
    res = run_bass_kernel_spmd(nc, in_maps, core_ids=list(range(NC)))
    out = np.concatenate([res.results[c]["out"] for c in range(NC)], axis=0)
    return out.astype(np.float32)



# revision 2
# speedup vs baseline: 1.2800x; 1.2800x over previous
"""BiGRU Trainium2 kernel (Bass/Tile), SPMD over 8 NeuronCores.

Sharding: one GRU direction per core (cores 0-3 fwd, 4-7 bwd; same program,
different data), 32 batch rows per core. Each core emits the partial
fc dot-product for its direction; host adds fwd+bwd partials and applies
the final sigmoid (128 scalar ops).

Layout ("gate-as-M"): hidden state and all gate tensors live as
[H-on-partitions, batch-on-free] tiles ([128, 4, 32] per 512-channel
quantity). The scan matmul makes the *weights* the stationary operand
(lhsT = Whh.T chunk [128,128]) and h the moving operand ([128,32], 32-row
stream): 48 matmuls/step at ~20ns each (fast weight load overlaps the
short stream). Benefits vs batch-as-M:
  - elementwise/activation tiles use all 128 partitions (4x fewer cycles)
  - gate biases enter via tiny indicator matmuls, not vector ops
  - h' is produced directly in next step's moving-operand layout (no
    transposes)

x-projection is hoisted: per 4-step block, 24 matmuls (M=128) compute
xp = Wih.T@x for all 3 gates directly *into the PSUM accumulator* that the
scan's r/z h-projections then accumulate onto (start=False) - the r/z gate
pre-activations never touch a vector engine until the fused sigmoid.
The n gate's xp stays separate (needed as xn + r*hn), in the same block.

z is computed as z' = 1-z = sigmoid(-(...)) by pre-negating z weights/bias
on the host; h' = z'*n + (1-z')*h with (1-z') and (1-z')*h computed
off-critical-path (hidden under tanh).
"""

import numpy as np
import ml_dtypes

import concourse.bass as bass
import concourse.bacc as bacc
import concourse.mybir as mybir
from concourse import tile
from concourse.bass_utils import run_bass_kernel_spmd

BF = ml_dtypes.bfloat16
V, E, H = 50000, 256, 512
B, T = 128, 512
NC = 8
BL = 32               # batch rows per core (one direction per core)
G = 3 * H             # 1536
EK = E // 128         # 2
HK = H // 128         # 4
NG = G // 128         # 12 gate chunks: r 0-3, z 4-7, n 8-11
TP = T + 8            # padded steps so the xp pipeline can run ahead
U = 32                # steps unrolled per hardware-loop iteration

bf = mybir.dt.bfloat16
f32 = mybir.dt.float32
ACT = mybir.ActivationFunctionType
ALU = mybir.AluOpType


def _build_nc():
    nc = bacc.Bacc(None, target_bir_lowering=False)

    xT = nc.dram_tensor("xT", [EK, 128, TP * BL], bf, kind="ExternalInput")
    whh = nc.dram_tensor("whh", [HK, 128, G], bf, kind="ExternalInput")
    wih = nc.dram_tensor("wih", [EK, 128, G], bf, kind="ExternalInput")
    biasx = nc.dram_tensor("biasx", [NG, 128], bf, kind="ExternalInput")
    bhn = nc.dram_tensor("bhn", [HK, 128], bf, kind="ExternalInput")
    indx = nc.dram_tensor("indx", [NG, NG * 128], bf, kind="ExternalInput")
    indh = nc.dram_tensor("indh", [HK, HK * BL], bf, kind="ExternalInput")
    fcw = nc.dram_tensor("fcw", [128, HK], bf, kind="ExternalInput")
    ones = nc.dram_tensor("ones", [1, BL], bf, kind="ExternalInput")
    yout = nc.dram_tensor("y", [1, BL], f32, kind="ExternalOutput")

    with tile.TileContext(nc) as tc:
        with (
            tc.tile_pool(name="cst", bufs=1) as cst,
            tc.tile_pool(name="wk", bufs=3) as wk,
            tc.tile_pool(name="ps", bufs=2, space="PSUM") as ps,
        ):
            # ---- resident SBUF constants ----
            xT_sb = cst.tile([128, EK, TP * BL], bf, tag="xT")
            for e in range(EK):
                nc.sync.dma_start(xT_sb[:, e, :], xT[e])
            whh_sb = cst.tile([128, HK, G], bf, tag="whh")
            for k in range(HK):
                nc.sync.dma_start(whh_sb[:, k, :], whh[k])
            wih_sb = cst.tile([128, EK, G], bf, tag="wih")
            for e in range(EK):
                nc.sync.dma_start(wih_sb[:, e, :], wih[e])
            biasx_sb = cst.tile([NG, 128], bf, tag="biasx")
            nc.sync.dma_start(biasx_sb[:, :], biasx[:, :])
            bhn_sb = cst.tile([HK, 128], bf, tag="bhn")
            nc.sync.dma_start(bhn_sb[:, :], bhn[:, :])
            indx_sb = cst.tile([NG, NG * 128], bf, tag="indx")
            nc.sync.dma_start(indx_sb[:, :], indx[:, :])
            indh_sb = cst.tile([HK, HK * BL], bf, tag="indh")
            nc.sync.dma_start(indh_sb[:, :], indh[:, :])
            fcw_sb = cst.tile([128, HK], bf, tag="fcw")
            nc.sync.dma_start(fcw_sb[:, :], fcw[:, :])
            ones_sb = cst.tile([1, BL], bf, tag="ones")
            nc.sync.dma_start(ones_sb[:, :], ones[:, :])

            # hidden state, ping-ponged per step: [128, HK, BL] bf16
            h_sb = [cst.tile([128, HK, BL], bf, tag=f"h{i}", name=f"h{i}")
                    for i in range(2)]
            nc.vector.memzero(h_sb[0][:, :, :])

            # ---- warmup: absorb each input DMA's completion wait into its
            # own trivial PE matmul (in-loop matmuls must not carry >1
            # outstanding dependency: the Ldweights uinstruction has a
            # single sync-wait slot) ----
            warm_ps = ps.tile([128, HK * BL], f32, tag="nh", name="warm")
            touches = (
                [xT_sb[0:1, e, 0:BL] for e in range(EK)]
                + [whh_sb[0:1, k, 0:BL] for k in range(HK)]
                + [wih_sb[0:1, e, 0:BL] for e in range(EK)]
                + [biasx_sb[0:1, 0:BL], bhn_sb[0:1, 0:BL],
                   indx_sb[0:1, 0:BL], indh_sb[0:1, 0:BL],
                   fcw_sb[0:1, 0:HK]]
            )
            first = True
            for src in touches:
                nc.tensor.matmul(warm_ps[0:1, 0:src.free_size()],
                                 ones_sb[:, 0:1], src,
                                 start=first, stop=False)
                first = False
            nc.tensor.matmul(warm_ps[0:1, 0:1], ones_sb[:, 0:1],
                             ones_sb[:, 0:1], start=False, stop=True)
            scrap = cst.tile([1, BL], bf, tag="scrap")
            nc.gpsimd.tensor_copy(scrap[0:1, :], xT_sb[0:1, 0, 0:BL])
            nc.scalar.activation(scrap[0:1, :], scrap[0:1, :], ACT.Sigmoid)
            nc.vector.tensor_copy(scrap[0:1, :], xT_sb[0:1, 1, 0:BL])

            # ---- xp block production: gates for steps [b4, b4+4) ----
            # psum block [128, NG, 4*BL] f32 (3 banks); 2 rotating buffers.
            def xp_block(t0):
                """Emit bias + x-projection into a fresh psum block; returns
                the block tile. t0 may be a register expression."""
                blk = ps.tile([128, NG * 128], f32, tag="xpA", name="xp")
                # all 12 chunk biases via K=12 indicator matmuls (<=512
                # moving elements per matmul: one per PSUM bank)
                for g in range(3):
                    nc.tensor.matmul(blk[:, g * 512:(g + 1) * 512],
                                     biasx_sb[:, :],
                                     indx_sb[:, g * 512:(g + 1) * 512],
                                     start=True, stop=False)
                for c in range(NG):
                    for e in range(EK):
                        nc.tensor.matmul(
                            blk[:, c * 128:(c + 1) * 128],
                            wih_sb[:, e, c * 128:(c + 1) * 128],
                            xT_sb[:, e, bass.ds(t0 * BL, 4 * BL)],
                            start=False, stop=(e == EK - 1),
                            skip_group_check=True)
                return blk

            def step(blk, s, h_cur, h_nxt):
                """One GRU step: consumes column s (0..3) of psum block."""
                co = s * BL  # column offset of this step inside the block

                # h-projection for r,z accumulates straight onto xp+bias
                for c in range(8):
                    for k in range(HK):
                        nc.tensor.matmul(
                            blk[:, c * 128 + co: c * 128 + co + BL],
                            whh_sb[:, k, c * 128:(c + 1) * 128],
                            h_cur[:, k, :],
                            start=False, stop=(k == HK - 1),
                            skip_group_check=True)
                # n-gate h-projection (kept apart from xp_n): bias then accum
                nh = ps.tile([128, HK * BL], f32, tag="nh", name="nh")
                nc.tensor.matmul(nh[:, :], bhn_sb[:, :], indh_sb[:, :],
                                 start=True, stop=False)
                for c in range(HK):
                    for k in range(HK):
                        nc.tensor.matmul(
                            nh[:, c * BL:(c + 1) * BL],
                            whh_sb[:, k, (8 + c) * 128:(9 + c) * 128],
                            h_cur[:, k, :],
                            start=False, stop=(k == HK - 1),
                            skip_group_check=True)

                # fused sigmoid over all r,z chunks -> bf16
                rz = wk.tile([128, 8, BL], bf, tag="rz", name="rz")
                nc.scalar.activation(
                    rz[:, :, :],
                    blk[:, :].rearrange("p (c n) -> p c n", c=NG)[:, 0:8, co:co + BL],
                    ACT.Sigmoid)
                # off-chain: z = 1 - z'; b2 = z*h   (hide under n-chain)
                # (GPSIMD/Pool has no PSUM access: all PSUM readers go to
                # DVE or ACT; all-SBUF bf16 ops go to Pool/DVE-4x.)
                zf = wk.tile([128, HK, BL], bf, tag="zf", name="zf")
                nc.gpsimd.tensor_scalar(zf[:, :, :], rz[:, 4:8, :], -1.0, 1.0,
                                        ALU.mult, ALU.add)
                b2 = wk.tile([128, HK, BL], bf, tag="b2", name="b2")
                nc.gpsimd.tensor_mul(b2[:, :, :], zf[:, :, :], h_cur[:, :, :])
                # n chain: t2 = r * (hn + bhn); v = t2 + xp_n; n = tanh(v)
                t2 = wk.tile([128, HK, BL], bf, tag="t2", name="t2")
                nc.vector.tensor_mul(
                    t2[:, :, :], rz[:, 0:4, :],
                    nh[:, :].rearrange("p (c n) -> p c n", c=HK))
                v = wk.tile([128, HK, BL], bf, tag="v", name="v")
                nc.vector.tensor_add(
                    v[:, :, :], t2[:, :, :],
                    blk[:, :].rearrange("p (c n) -> p c n", c=NG)[:, 8:12, co:co + BL])
                n_t = wk.tile([128, HK, BL], bf, tag="n", name="n")
                nc.scalar.activation(n_t[:, :, :], v[:, :, :], ACT.Tanh)
                # h' = z'*n + z*h
                a_t = wk.tile([128, HK, BL], bf, tag="a", name="a")
                nc.gpsimd.tensor_mul(a_t[:, :, :], rz[:, 4:8, :], n_t[:, :, :])
                nc.vector.tensor_add(h_nxt[:, :, :], a_t[:, :, :], b2[:, :, :])

            # prologue: produce block 0 (steps 0..3)
            blk_cur = xp_block(0)

            ASSUMED_EVEN_BLOCKS = (U // 4) % 2 == 0
            assert ASSUMED_EVEN_BLOCKS and U % 4 == 0

            with tc.For_i(0, T // U, 1, staggered_reset=True,
                          hint_engines=(mybir.EngineType.PE,)) as it:
                t_base = it * U
                for u4 in range(U // 4):
                    # produce the NEXT block, then run this block's 4 steps
                    blk_nxt = xp_block(t_base + u4 * 4 + 4)
                    for s in range(4):
                        t = u4 * 4 + s          # static step index in body
                        step(blk_cur, s, h_sb[t % 2], h_sb[(t + 1) % 2])
                    blk_cur = blk_nxt

            # ---- fc partial: y[b] = sum_k fcw[:,k] . h[:,k,b] ----
            fc_ps = ps.tile([1, BL], f32, tag="nh", name="fc")
            for k in range(HK):
                nc.tensor.matmul(fc_ps[:, :], fcw_sb[:, k:k + 1],
                                 h_sb[T % 2][:, k, :],
                                 start=(k == 0), stop=(k == HK - 1))
            y_sb = cst.tile([1, BL], f32, tag="y")
            nc.scalar.copy(y_sb[:, :], fc_ps[:, :])
            nc.sync.dma_start(yout[:, :], y_sb[:, :])
    nc.finalize()
    return nc


_NC_CACHE = None


def _get_nc():
    global _NC_CACHE
    if _NC_CACHE is None:
        _NC_CACHE = _build_nc()
    return _NC_CACHE


def _prep_xT(x_c, rev):
    """x_c [BL, T, E] f32 -> [EK, 128, TP*BL] bf16 (optionally reversed)."""
    if rev:
        x_c = x_c[:, ::-1, :]
    xt = np.zeros((EK, 128, TP * BL), np.float32)
    # xt[e, p, t*BL + b] = x_c[b, t, 128e + p]
    xt[:, :, :T * BL] = np.ascontiguousarray(
        x_c.transpose(2, 1, 0)).reshape(EK, 128, T * BL)
    return xt.astype(BF)


def _prep_weights(W_ih, W_hh, b_ih, b_hh):
    Wi = np.array(W_ih, np.float32).copy()
    Wh = np.array(W_hh, np.float32).copy()
    Wi[H:2 * H] *= -1.0
    Wh[H:2 * H] *= -1.0
    # whh[k] = Whh'[:, 128k:128k+128].T  -> [128, G]
    whhT = np.ascontiguousarray(Wh.T).reshape(HK, 128, G).astype(BF)
    wihT = np.ascontiguousarray(Wi.T).reshape(EK, 128, G).astype(BF)
    bi = np.asarray(b_ih, np.float32)
    bh = np.asarray(b_hh, np.float32)
    vb = np.concatenate([
        bi[0:H] + bh[0:H],
        -(bi[H:2 * H] + bh[H:2 * H]),
        bi[2 * H:3 * H]])
    biasx = vb.reshape(NG, 128).astype(BF)
    bhn_ = bh[2 * H:3 * H].reshape(HK, 128).astype(BF)
    return wihT, whhT, biasx, bhn_


def prepare_in_maps(inputs, emb, W_ih_f, W_hh_f, b_ih_f, b_hh_f,
                    W_ih_b, W_hh_b, b_ih_b, b_hh_b, fc_w, fc_b):
    ids = np.asarray(inputs)
    emb = np.asarray(emb, np.float32)
    x = emb[ids]  # [B, T, E]

    indx_ = np.zeros((NG, NG * 128), np.float32)
    for k in range(NG):
        indx_[k, k * 128:(k + 1) * 128] = 1.0
    indh_ = np.zeros((HK, HK * BL), np.float32)
    for k in range(HK):
        indh_[k, k * BL:(k + 1) * BL] = 1.0
    fc = np.asarray(fc_w, np.float32)[0]
    ones = np.ones((1, BL), np.float32)

    per_dir = {}
    for d, (Wi, Wh, bi, bh) in (
            ("f", (W_ih_f, W_hh_f, b_ih_f, b_hh_f)),
            ("b", (W_ih_b, W_hh_b, b_ih_b, b_hh_b))):
        wihT, whhT, biasx, bhn_ = _prep_weights(Wi, Wh, bi, bh)
        off = 0 if d == "f" else H
        fcw_ = fc[off:off + H].reshape(HK, 128).T.copy().astype(BF)  # [128, HK]
        per_dir[d] = dict(whh=whhT, wih=wihT, biasx=biasx, bhn=bhn_,
                          fcw=fcw_, indx=indx_.astype(BF),
                          indh=indh_.astype(BF), ones=ones.astype(BF))

    in_maps = []
    for c in range(NC):
        d = "f" if c < 4 else "b"
        sh = c % 4
        x_c = x[sh * BL:(sh + 1) * BL]
        in_maps.append(dict(per_dir[d], xT=_prep_xT(x_c, d == "b")))
    return in_maps


def kernel(**inputs):
    in_maps = prepare_in_maps(**inputs)
    nc = _get_nc()
    res = run_bass_kernel_spmd(nc, in_maps, core_ids=list(range(NC)))
    fcb = np.float32(np.asarray(inputs["fc_b"], np.float32).reshape(-1)[0])
    out = np.empty((B, 1), np.float32)
    for sh in range(4):
        yf = np.asarray(res.results[sh]["y"], np.float32).reshape(BL)
        yb = np.asarray(res.results[4 + sh]["y"], np.float32).reshape(BL)
        out[sh * BL:(sh + 1) * BL, 0] = 1.0 / (1.0 + np.exp(-(yf + yb + fcb)))
    return out


# revision 3
# speedup vs baseline: 1.2956x; 1.0122x over previous
"""BiGRU Trainium2 kernel (Bass/Tile), SPMD over 8 NeuronCores.

Sharding: one GRU direction per core (cores 0-3 fwd, 4-7 bwd; same program,
different data), 32 batch rows per core. Each core emits the partial
fc dot-product for its direction; host adds fwd+bwd partials and applies
the final sigmoid (128 scalar ops).

Layout ("gate-as-M"): hidden state and all gate tensors live as
[H-on-partitions, batch-on-free] tiles ([128, 4, 32] per 512-channel
quantity). The scan matmul makes the *weights* the stationary operand
(lhsT = Whh.T chunk [128,128]) and h the moving operand ([128,32], 32-row
stream): 48 matmuls/step at ~20ns each (fast weight load overlaps the
short stream). Benefits vs batch-as-M:
  - elementwise/activation tiles use all 128 partitions (4x fewer cycles)
  - gate biases enter via tiny indicator matmuls, not vector ops
  - h' is produced directly in next step's moving-operand layout (no
    transposes)

x-projection is hoisted: per 4-step block, 24 matmuls (M=128) compute
xp = Wih.T@x for all 3 gates directly *into the PSUM accumulator* that the
scan's r/z h-projections then accumulate onto (start=False) - the r/z gate
pre-activations never touch a vector engine until the fused sigmoid.
The n gate's xp stays separate (needed as xn + r*hn), in the same block.

z is computed as z' = 1-z = sigmoid(-(...)) by pre-negating z weights/bias
on the host; h' = z'*n + (1-z')*h with (1-z') and (1-z')*h computed
off-critical-path (hidden under tanh).
"""

import numpy as np
import ml_dtypes

import concourse.bass as bass
import concourse.bacc as bacc
import concourse.mybir as mybir
from concourse import tile
from concourse.bass_utils import run_bass_kernel_spmd

BF = ml_dtypes.bfloat16
V, E, H = 50000, 256, 512
B, T = 128, 512
NC = 8
BL = 32               # batch rows per core (one direction per core)
G = 3 * H             # 1536
EK = E // 128         # 2
HK = H // 128         # 4
NG = G // 128         # 12 gate chunks: r 0-3, z 4-7, n 8-11
TP = T + 8            # padded steps so the xp pipeline can run ahead
U = 32                # steps unrolled per hardware-loop iteration

bf = mybir.dt.bfloat16
f32 = mybir.dt.float32
ACT = mybir.ActivationFunctionType
ALU = mybir.AluOpType


def _build_nc():
    nc = bacc.Bacc(None, target_bir_lowering=False)

    xT = nc.dram_tensor("xT", [EK, 128, TP * BL], bf, kind="ExternalInput")
    whh = nc.dram_tensor("whh", [HK, 128, G], bf, kind="ExternalInput")
    wih = nc.dram_tensor("wih", [EK, 128, G], bf, kind="ExternalInput")
    biasx = nc.dram_tensor("biasx", [NG, 128], bf, kind="ExternalInput")
    bhn = nc.dram_tensor("bhn", [HK, 128], bf, kind="ExternalInput")
    indx = nc.dram_tensor("indx", [NG, NG * 128], bf, kind="ExternalInput")
    indh = nc.dram_tensor("indh", [HK, HK * BL], bf, kind="ExternalInput")
    fcw = nc.dram_tensor("fcw", [128, HK], bf, kind="ExternalInput")
    ones = nc.dram_tensor("ones", [1, BL], bf, kind="ExternalInput")
    yout = nc.dram_tensor("y", [1, BL], f32, kind="ExternalOutput")

    with tile.TileContext(nc) as tc:
        with (
            tc.tile_pool(name="cst", bufs=1) as cst,
            tc.tile_pool(name="wk", bufs=3) as wk,
            tc.tile_pool(name="ps", bufs=2, space="PSUM") as ps,
        ):
            # ---- resident SBUF constants ----
            xT_sb = cst.tile([128, EK, TP * BL], bf, tag="xT")
            for e in range(EK):
                nc.sync.dma_start(xT_sb[:, e, :], xT[e])
            whh_sb = cst.tile([128, HK, G], bf, tag="whh")
            for k in range(HK):
                nc.sync.dma_start(whh_sb[:, k, :], whh[k])
            wih_sb = cst.tile([128, EK, G], bf, tag="wih")
            for e in range(EK):
                nc.sync.dma_start(wih_sb[:, e, :], wih[e])
            biasx_sb = cst.tile([NG, 128], bf, tag="biasx")
            nc.sync.dma_start(biasx_sb[:, :], biasx[:, :])
            bhn_sb = cst.tile([HK, 128], bf, tag="bhn")
            nc.sync.dma_start(bhn_sb[:, :], bhn[:, :])
            indx_sb = cst.tile([NG, NG * 128], bf, tag="indx")
            nc.sync.dma_start(indx_sb[:, :], indx[:, :])
            indh_sb = cst.tile([HK, HK * BL], bf, tag="indh")
            nc.sync.dma_start(indh_sb[:, :], indh[:, :])
            fcw_sb = cst.tile([128, HK], bf, tag="fcw")
            nc.sync.dma_start(fcw_sb[:, :], fcw[:, :])
            ones_sb = cst.tile([1, BL], bf, tag="ones")
            nc.sync.dma_start(ones_sb[:, :], ones[:, :])

            # hidden state, ping-ponged per step: [128, HK, BL] bf16
            h_sb = [cst.tile([128, HK, BL], bf, tag=f"h{i}", name=f"h{i}")
                    for i in range(2)]
            nc.vector.memzero(h_sb[0][:, :, :])

            # ---- warmup: absorb each input DMA's completion wait into its
            # own trivial PE matmul (in-loop matmuls must not carry >1
            # outstanding dependency: the Ldweights uinstruction has a
            # single sync-wait slot) ----
            warm_ps = ps.tile([128, HK * BL], f32, tag="nh", name="warm")
            touches = (
                [xT_sb[0:1, e, 0:BL] for e in range(EK)]
                + [whh_sb[0:1, k, 0:BL] for k in range(HK)]
                + [wih_sb[0:1, e, 0:BL] for e in range(EK)]
                + [biasx_sb[0:1, 0:BL], bhn_sb[0:1, 0:BL],
                   indx_sb[0:1, 0:BL], indh_sb[0:1, 0:BL],
                   fcw_sb[0:1, 0:HK]]
            )
            first = True
            for src in touches:
                nc.tensor.matmul(warm_ps[0:1, 0:src.free_size()],
                                 ones_sb[:, 0:1], src,
                                 start=first, stop=False)
                first = False
            nc.tensor.matmul(warm_ps[0:1, 0:1], ones_sb[:, 0:1],
                             ones_sb[:, 0:1], start=False, stop=True)
            scrap = cst.tile([1, BL], bf, tag="scrap")
            nc.gpsimd.tensor_copy(scrap[0:1, :], xT_sb[0:1, 0, 0:BL])
            nc.scalar.activation(scrap[0:1, :], scrap[0:1, :], ACT.Sigmoid)
            nc.vector.tensor_copy(scrap[0:1, :], xT_sb[0:1, 1, 0:BL])

            # ---- xp block production: gates for steps [b4, b4+4) ----
            # psum block [128, NG, 4*BL] f32 (3 banks); 2 rotating buffers.
            def xp_block(t0):
                """Emit bias + x-projection into a fresh psum block; returns
                the block tile. t0 may be a register expression. x is staged
                through a static SBUF buffer (GPSIMD copy) so every in-loop
                PE instruction has register-free access patterns."""
                xs = wk.tile([128, EK, 4 * BL], bf, tag="xs", name="xs")
                for e in range(EK):
                    nc.gpsimd.tensor_copy(xs[:, e, :],
                                          xT_sb[:, e, bass.ds(t0 * BL, 4 * BL)])
                blk = ps.tile([128, NG * 128], f32, tag="xpA", name="xp")
                # all 12 chunk biases via K=12 indicator matmuls (<=512
                # moving elements per matmul: one per PSUM bank)
                for g in range(3):
                    nc.tensor.matmul(blk[:, g * 512:(g + 1) * 512],
                                     biasx_sb[:, :],
                                     indx_sb[:, g * 512:(g + 1) * 512],
                                     start=True, stop=False)
                for c in range(NG):
                    for e in range(EK):
                        nc.tensor.matmul(
                            blk[:, c * 128:(c + 1) * 128],
                            wih_sb[:, e, c * 128:(c + 1) * 128],
                            xs[:, e, :],
                            start=False, stop=(e == EK - 1),
                            skip_group_check=True)
                return blk

            def step(blk, s, h_cur, h_nxt):
                """One GRU step: consumes column s (0..3) of psum block."""
                co = s * BL  # column offset of this step inside the block

                # h-projection for r,z accumulates straight onto xp+bias
                for c in range(8):
                    for k in range(HK):
                        nc.tensor.matmul(
                            blk[:, c * 128 + co: c * 128 + co + BL],
                            whh_sb[:, k, c * 128:(c + 1) * 128],
                            h_cur[:, k, :],
                            start=False, stop=(k == HK - 1),
                            skip_group_check=True)
                # n-gate h-projection (kept apart from xp_n): bias then accum
                nh = ps.tile([128, HK * BL], f32, tag="nh", name="nh")
                nc.tensor.matmul(nh[:, :], bhn_sb[:, :], indh_sb[:, :],
                                 start=True, stop=False)
                for c in range(HK):
                    for k in range(HK):
                        nc.tensor.matmul(
                            nh[:, c * BL:(c + 1) * BL],
                            whh_sb[:, k, (8 + c) * 128:(9 + c) * 128],
                            h_cur[:, k, :],
                            start=False, stop=(k == HK - 1),
                            skip_group_check=True)

                # fused sigmoid over all r,z chunks -> bf16
                rz = wk.tile([128, 8, BL], bf, tag="rz", name="rz")
                nc.scalar.activation(
                    rz[:, :, :],
                    blk[:, :].rearrange("p (c n) -> p c n", c=NG)[:, 0:8, co:co + BL],
                    ACT.Sigmoid)
                # off-chain: z = 1 - z'; b2 = z*h   (hide under n-chain)
                # (GPSIMD/Pool has no PSUM access: all PSUM readers go to
                # DVE or ACT; all-SBUF bf16 ops go to Pool/DVE-4x.)
                zf = wk.tile([128, HK, BL], bf, tag="zf", name="zf")
                nc.gpsimd.tensor_scalar(zf[:, :, :], rz[:, 4:8, :], -1.0, 1.0,
                                        ALU.mult, ALU.add)
                b2 = wk.tile([128, HK, BL], bf, tag="b2", name="b2")
                nc.gpsimd.tensor_mul(b2[:, :, :], zf[:, :, :], h_cur[:, :, :])
                # n chain: t2 = r * (hn + bhn); v = t2 + xp_n; n = tanh(v)
                t2 = wk.tile([128, HK, BL], bf, tag="t2", name="t2")
                nc.vector.tensor_mul(
                    t2[:, :, :], rz[:, 0:4, :],
                    nh[:, :].rearrange("p (c n) -> p c n", c=HK))
                v = wk.tile([128, HK, BL], bf, tag="v", name="v")
                nc.vector.tensor_add(
                    v[:, :, :], t2[:, :, :],
                    blk[:, :].rearrange("p (c n) -> p c n", c=NG)[:, 8:12, co:co + BL])
                n_t = wk.tile([128, HK, BL], bf, tag="n", name="n")
                nc.scalar.activation(n_t[:, :, :], v[:, :, :], ACT.Tanh)
                # h' = z'*n + z*h
                a_t = wk.tile([128, HK, BL], bf, tag="a", name="a")
                nc.gpsimd.tensor_mul(a_t[:, :, :], rz[:, 4:8, :], n_t[:, :, :])
                nc.vector.tensor_add(h_nxt[:, :, :], a_t[:, :, :], b2[:, :, :])

            # prologue: produce block 0 (steps 0..3)
            blk_cur = xp_block(0)

            ASSUMED_EVEN_BLOCKS = (U // 4) % 2 == 0
            assert ASSUMED_EVEN_BLOCKS and U % 4 == 0

            with tc.For_i(0, T // U, 1, staggered_reset=True,
                          hint_engines=(mybir.EngineType.PE,)) as it:
                t_base = it * U
                for u4 in range(U // 4):
                    # produce the NEXT block, then run this block's 4 steps
                    blk_nxt = xp_block(t_base + u4 * 4 + 4)
                    for s in range(4):
                        t = u4 * 4 + s          # static step index in body
                        step(blk_cur, s, h_sb[t % 2], h_sb[(t + 1) % 2])
                    blk_cur = blk_nxt

            # ---- fc partial: y[b] = sum_k fcw[:,k] . h[:,k,b] ----
            fc_ps = ps.tile([1, BL], f32, tag="nh", name="fc")
            for k in range(HK):
                nc.tensor.matmul(fc_ps[:, :], fcw_sb[:, k:k + 1],
                                 h_sb[T % 2][:, k, :],
                                 start=(k == 0), stop=(k == HK - 1))
            y_sb = cst.tile([1, BL], f32, tag="y")
            nc.scalar.copy(y_sb[:, :], fc_ps[:, :])
            nc.sync.dma_start(yout[:, :], y_sb[:, :])
    nc.finalize()
    return nc


_NC_CACHE = None


def _get_nc():
    global _NC_CACHE
    if _NC_CACHE is None:
        _NC_CACHE = _build_nc()
    return _NC_CACHE


def _prep_xT(x_c, rev):
    """x_c [BL, T, E] f32 -> [EK, 128, TP*BL] bf16 (optionally reversed)."""
    if rev:
        x_c = x_c[:, ::-1, :]
    xt = np.zeros((EK, 128, TP * BL), np.float32)
    # xt[e, p, t*BL + b] = x_c[b, t, 128e + p]
    xt[:, :, :T * BL] = np.ascontiguousarray(
        x_c.transpose(2, 1, 0)).reshape(EK, 128, T * BL)
    return xt.astype(BF)


def _prep_weights(W_ih, W_hh, b_ih, b_hh):
    Wi = np.array(W_ih, np.float32).copy()
    Wh = np.array(W_hh, np.float32).copy()
    Wi[H:2 * H] *= -1.0
    Wh[H:2 * H] *= -1.0
    # whh[k] = Whh'[:, 128k:128k+128].T  -> [128, G]
    whhT = np.ascontiguousarray(Wh.T).reshape(HK, 128, G).astype(BF)
    wihT = np.ascontiguousarray(Wi.T).reshape(EK, 128, G).astype(BF)
    bi = np.asarray(b_ih, np.float32)
    bh = np.asarray(b_hh, np.float32)
    vb = np.concatenate([
        bi[0:H] + bh[0:H],
        -(bi[H:2 * H] + bh[H:2 * H]),
        bi[2 * H:3 * H]])
    biasx = vb.reshape(NG, 128).astype(BF)
    bhn_ = bh[2 * H:3 * H].reshape(HK, 128).astype(BF)
    return wihT, whhT, biasx, bhn_


def prepare_in_maps(inputs, emb, W_ih_f, W_hh_f, b_ih_f, b_hh_f,
                    W_ih_b, W_hh_b, b_ih_b, b_hh_b, fc_w, fc_b):
    ids = np.asarray(inputs)
    emb = np.asarray(emb, np.float32)
    x = emb[ids]  # [B, T, E]

    indx_ = np.zeros((NG, NG * 128), np.float32)
    for k in range(NG):
        indx_[k, k * 128:(k + 1) * 128] = 1.0
    indh_ = np.zeros((HK, HK * BL), np.float32)
    for k in range(HK):
        indh_[k, k * BL:(k + 1) * BL] = 1.0
    fc = np.asarray(fc_w, np.float32)[0]
    ones = np.ones((1, BL), np.float32)

    per_dir = {}
    for d, (Wi, Wh, bi, bh) in (
            ("f", (W_ih_f, W_hh_f, b_ih_f, b_hh_f)),
            ("b", (W_ih_b, W_hh_b, b_ih_b, b_hh_b))):
        wihT, whhT, biasx, bhn_ = _prep_weights(Wi, Wh, bi, bh)
        off = 0 if d == "f" else H
        fcw_ = fc[off:off + H].reshape(HK, 128).T.copy().astype(BF)  # [128, HK]
        per_dir[d] = dict(whh=whhT, wih=wihT, biasx=biasx, bhn=bhn_,
                          fcw=fcw_, indx=indx_.astype(BF),
                          indh=indh_.astype(BF), ones=ones.astype(BF))

    in_maps = []
    for c in range(NC):
        d = "f" if c < 4 else "b"
        sh = c % 4
        x_c = x[sh * BL:(sh + 1) * BL]
        in_maps.append(dict(per_dir[d], xT=_prep_xT(x_c, d == "b")))
    return in_maps


def kernel(**inputs):
    in_maps = prepare_in_maps(**inputs)
    nc = _get_nc()
    res = run_bass_kernel_spmd(nc, in_maps, core_ids=list(range(NC)))
    fcb = np.float32(np.asarray(inputs["fc_b"], np.float32).reshape(-1)[0])
    out = np.empty((B, 1), np.float32)
    for sh in range(4):
        yf = np.asarray(res.results[sh]["y"], np.float32).reshape(BL)
        yb = np.asarray(res.results[4 + sh]["y"], np.float32).reshape(BL)
        out[sh * BL:(sh + 1) * BL, 0] = 1.0 / (1.0 + np.exp(-(yf + yb + fcb)))
    return out


# revision 4
# speedup vs baseline: 1.3070x; 1.0088x over previous
"""BiGRU Trainium2 kernel (Bass/Tile), SPMD over 8 NeuronCores.

Sharding: one GRU direction per core (cores 0-3 fwd, 4-7 bwd; same program,
different data), 32 batch rows per core. Each core emits the partial
fc dot-product for its direction; host adds fwd+bwd partials and applies
the final sigmoid (128 scalar ops).

Layout ("gate-as-M"): hidden state and all gate tensors live as
[H-on-partitions, batch-on-free] tiles ([128, 4, 32] per 512-channel
quantity). The scan matmul makes the *weights* the stationary operand
(lhsT = Whh.T chunk [128,128]) and h the moving operand ([128,32], 32-row
stream): 48 matmuls/step at ~20ns each (fast weight load overlaps the
short stream). Benefits vs batch-as-M:
  - elementwise/activation tiles use all 128 partitions (4x fewer cycles)
  - gate biases enter via tiny indicator matmuls, not vector ops
  - h' is produced directly in next step's moving-operand layout (no
    transposes)

x-projection is hoisted: per 4-step block, 24 matmuls (M=128) compute
xp = Wih.T@x for all 3 gates directly *into the PSUM accumulator* that the
scan's r/z h-projections then accumulate onto (start=False) - the r/z gate
pre-activations never touch a vector engine until the fused sigmoid.
The n gate's xp stays separate (needed as xn + r*hn), in the same block.

z is computed as z' = 1-z = sigmoid(-(...)) by pre-negating z weights/bias
on the host; h' = z'*n + (1-z')*h with (1-z') and (1-z')*h computed
off-critical-path (hidden under tanh).
"""

import numpy as np
import ml_dtypes

import concourse.bass as bass
import concourse.bacc as bacc
import concourse.mybir as mybir
from concourse import tile
from concourse.bass_utils import run_bass_kernel_spmd

BF = ml_dtypes.bfloat16
V, E, H = 50000, 256, 512
B, T = 128, 512
NC = 8
BL = 32               # batch rows per core (one direction per core)
G = 3 * H             # 1536
EK = E // 128         # 2
HK = H // 128         # 4
NG = G // 128         # 12 gate chunks: r 0-3, z 4-7, n 8-11
TP = T + 8            # padded steps so the xp pipeline can run ahead
U = 32                # steps unrolled per hardware-loop iteration

bf = mybir.dt.bfloat16
f32 = mybir.dt.float32
ACT = mybir.ActivationFunctionType
ALU = mybir.AluOpType


def _build_nc():
    nc = bacc.Bacc(None, target_bir_lowering=False)

    xT = nc.dram_tensor("xT", [EK, 128, TP * BL], bf, kind="ExternalInput")
    whh = nc.dram_tensor("whh", [HK, 128, G], bf, kind="ExternalInput")
    wih = nc.dram_tensor("wih", [EK, 128, G], bf, kind="ExternalInput")
    biasx = nc.dram_tensor("biasx", [NG, 128], bf, kind="ExternalInput")
    bhn = nc.dram_tensor("bhn", [HK, 128], bf, kind="ExternalInput")
    indx = nc.dram_tensor("indx", [NG, NG * 128], bf, kind="ExternalInput")
    indh = nc.dram_tensor("indh", [HK, HK * BL], bf, kind="ExternalInput")
    fcw = nc.dram_tensor("fcw", [128, HK], bf, kind="ExternalInput")
    ones = nc.dram_tensor("ones", [1, BL], bf, kind="ExternalInput")
    yout = nc.dram_tensor("y", [1, BL], f32, kind="ExternalOutput")

    with tile.TileContext(nc) as tc:
        with (
            tc.tile_pool(name="cst", bufs=1) as cst,
            tc.tile_pool(name="wk", bufs=3) as wk,
            tc.tile_pool(name="ps", bufs=2, space="PSUM") as ps,
        ):
            # ---- resident SBUF constants ----
            xT_sb = cst.tile([128, EK, TP * BL], bf, tag="xT")
            for e in range(EK):
                nc.sync.dma_start(xT_sb[:, e, :], xT[e])
            whh_sb = cst.tile([128, HK, G], bf, tag="whh")
            for k in range(HK):
                nc.sync.dma_start(whh_sb[:, k, :], whh[k])
            wih_sb = cst.tile([128, EK, G], bf, tag="wih")
            for e in range(EK):
                nc.sync.dma_start(wih_sb[:, e, :], wih[e])
            biasx_sb = cst.tile([NG, 128], bf, tag="biasx")
            nc.sync.dma_start(biasx_sb[:, :], biasx[:, :])
            bhn_sb = cst.tile([HK, 128], bf, tag="bhn")
            nc.sync.dma_start(bhn_sb[:, :], bhn[:, :])
            indx_sb = cst.tile([NG, NG * 128], bf, tag="indx")
            nc.sync.dma_start(indx_sb[:, :], indx[:, :])
            indh_sb = cst.tile([HK, HK * BL], bf, tag="indh")
            nc.sync.dma_start(indh_sb[:, :], indh[:, :])
            fcw_sb = cst.tile([128, HK], bf, tag="fcw")
            nc.sync.dma_start(fcw_sb[:, :], fcw[:, :])
            ones_sb = cst.tile([1, BL], bf, tag="ones")
            nc.sync.dma_start(ones_sb[:, :], ones[:, :])

            # hidden state, ping-ponged per step: [128, HK, BL] bf16
            h_sb = [cst.tile([128, HK, BL], bf, tag=f"h{i}", name=f"h{i}")
                    for i in range(4)]
            nc.vector.memzero(h_sb[0][:, :, :])

            # ---- warmup: absorb each input DMA's completion wait into its
            # own trivial PE matmul (in-loop matmuls must not carry >1
            # outstanding dependency: the Ldweights uinstruction has a
            # single sync-wait slot) ----
            warm_ps = ps.tile([128, HK * BL], f32, tag="nh", name="warm")
            touches = (
                [xT_sb[0:1, e, 0:BL] for e in range(EK)]
                + [whh_sb[0:1, k, 0:BL] for k in range(HK)]
                + [wih_sb[0:1, e, 0:BL] for e in range(EK)]
                + [biasx_sb[0:1, 0:BL], bhn_sb[0:1, 0:BL],
                   indx_sb[0:1, 0:BL], indh_sb[0:1, 0:BL],
                   fcw_sb[0:1, 0:HK]]
            )
            first = True
            for src in touches:
                nc.tensor.matmul(warm_ps[0:1, 0:src.free_size()],
                                 ones_sb[:, 0:1], src,
                                 start=first, stop=False)
                first = False
            nc.tensor.matmul(warm_ps[0:1, 0:1], ones_sb[:, 0:1],
                             ones_sb[:, 0:1], start=False, stop=True)
            scrap = cst.tile([1, BL], bf, tag="scrap")
            nc.gpsimd.tensor_copy(scrap[0:1, :], xT_sb[0:1, 0, 0:BL])
            nc.scalar.activation(scrap[0:1, :], scrap[0:1, :], ACT.Sigmoid)
            nc.vector.tensor_copy(scrap[0:1, :], xT_sb[0:1, 1, 0:BL])

            # ---- xp block production: gates for steps [b4, b4+4) ----
            # psum block [128, NG, 4*BL] f32 (3 banks); 2 rotating buffers.
            def xp_block(t0):
                """Emit bias + x-projection into a fresh psum block; returns
                the block tile. t0 may be a register expression. x is staged
                through a static SBUF buffer (GPSIMD copy) so every in-loop
                PE instruction has register-free access patterns."""
                xs = wk.tile([128, EK, 4 * BL], bf, tag="xs", name="xs")
                for e in range(EK):
                    nc.gpsimd.tensor_copy(xs[:, e, :],
                                          xT_sb[:, e, bass.ds(t0 * BL, 4 * BL)])
                blk = ps.tile([128, NG * 128], f32, tag="xpA", name="xp")
                # all 12 chunk biases via K=12 indicator matmuls (<=512
                # moving elements per matmul: one per PSUM bank)
                for g in range(3):
                    nc.tensor.matmul(blk[:, g * 512:(g + 1) * 512],
                                     biasx_sb[:, :],
                                     indx_sb[:, g * 512:(g + 1) * 512],
                                     start=True, stop=False)
                for c in range(NG):
                    for e in range(EK):
                        nc.tensor.matmul(
                            blk[:, c * 128:(c + 1) * 128],
                            wih_sb[:, e, c * 128:(c + 1) * 128],
                            xs[:, e, :],
                            start=False, stop=(e == EK - 1),
                            skip_group_check=True)
                return blk

            def step(blk, s, h_cur, h_nxt):
                """One GRU step: consumes column s (0..3) of psum block."""
                co = s * BL  # column offset of this step inside the block

                # h-projection for r,z accumulates straight onto xp+bias
                for c in range(8):
                    for k in range(HK):
                        nc.tensor.matmul(
                            blk[:, c * 128 + co: c * 128 + co + BL],
                            whh_sb[:, k, c * 128:(c + 1) * 128],
                            h_cur[:, k, :],
                            start=False, stop=(k == HK - 1),
                            skip_group_check=True)
                # n-gate h-projection (kept apart from xp_n): bias then accum
                nh = ps.tile([128, HK * BL], f32, tag="nh", name="nh")
                nc.tensor.matmul(nh[:, :], bhn_sb[:, :], indh_sb[:, :],
                                 start=True, stop=False)
                for c in range(HK):
                    for k in range(HK):
                        nc.tensor.matmul(
                            nh[:, c * BL:(c + 1) * BL],
                            whh_sb[:, k, (8 + c) * 128:(9 + c) * 128],
                            h_cur[:, k, :],
                            start=False, stop=(k == HK - 1),
                            skip_group_check=True)

                # fused sigmoid over all r,z chunks -> bf16
                rz = wk.tile([128, 8, BL], bf, tag="rz", name="rz")
                nc.scalar.activation(
                    rz[:, :, :],
                    blk[:, :].rearrange("p (c n) -> p c n", c=NG)[:, 0:8, co:co + BL],
                    ACT.Sigmoid)
                # off-chain: z = 1 - z'; b2 = z*h   (hide under n-chain)
                # (GPSIMD/Pool has no PSUM access: all PSUM readers go to
                # DVE or ACT; all-SBUF bf16 ops go to Pool/DVE-4x.)
                zf = wk.tile([128, HK, BL], bf, tag="zf", name="zf")
                nc.gpsimd.tensor_scalar(zf[:, :, :], rz[:, 4:8, :], -1.0, 1.0,
                                        ALU.mult, ALU.add)
                b2 = wk.tile([128, HK, BL], bf, tag="b2", name="b2")
                nc.gpsimd.tensor_mul(b2[:, :, :], zf[:, :, :], h_cur[:, :, :])
                # n chain: t2 = r * (hn + bhn); v = t2 + xp_n; n = tanh(v)
                t2 = wk.tile([128, HK, BL], bf, tag="t2", name="t2")
                nc.vector.tensor_mul(
                    t2[:, :, :], rz[:, 0:4, :],
                    nh[:, :].rearrange("p (c n) -> p c n", c=HK))
                v = wk.tile([128, HK, BL], bf, tag="v", name="v")
                nc.vector.tensor_add(
                    v[:, :, :], t2[:, :, :],
                    blk[:, :].rearrange("p (c n) -> p c n", c=NG)[:, 8:12, co:co + BL])
                n_t = wk.tile([128, HK, BL], bf, tag="n", name="n")
                nc.scalar.activation(n_t[:, :, :], v[:, :, :], ACT.Tanh)
                # h' = z'*n + z*h
                a_t = wk.tile([128, HK, BL], bf, tag="a", name="a")
                nc.gpsimd.tensor_mul(a_t[:, :, :], rz[:, 4:8, :], n_t[:, :, :])
                nc.vector.tensor_add(h_nxt[:, :, :], a_t[:, :, :], b2[:, :, :])

            # prologue: produce block 0 (steps 0..3)
            blk_cur = xp_block(0)

            ASSUMED_EVEN_BLOCKS = (U // 4) % 2 == 0
            assert ASSUMED_EVEN_BLOCKS and U % 4 == 0

            with tc.For_i(0, T // U, 1, staggered_reset=True,
                          hint_engines=(mybir.EngineType.PE,)) as it:
                t_base = it * U
                for u4 in range(U // 4):
                    # produce the NEXT block, then run this block's 4 steps
                    blk_nxt = xp_block(t_base + u4 * 4 + 4)
                    for s in range(4):
                        t = u4 * 4 + s          # static step index in body
                        step(blk_cur, s, h_sb[t % 4], h_sb[(t + 1) % 4])
                    blk_cur = blk_nxt

            # ---- fc partial: y[b] = sum_k fcw[:,k] . h[:,k,b] ----
            fc_ps = ps.tile([1, BL], f32, tag="nh", name="fc")
            for k in range(HK):
                nc.tensor.matmul(fc_ps[:, :], fcw_sb[:, k:k + 1],
                                 h_sb[T % 4][:, k, :],
                                 start=(k == 0), stop=(k == HK - 1))
            y_sb = cst.tile([1, BL], f32, tag="y")
            nc.scalar.copy(y_sb[:, :], fc_ps[:, :])
            nc.sync.dma_start(yout[:, :], y_sb[:, :])
    nc.finalize()
    return nc


_NC_CACHE = None


def _get_nc():
    global _NC_CACHE
    if _NC_CACHE is None:
        _NC_CACHE = _build_nc()
    return _NC_CACHE


def _prep_xT(x_c, rev):
    """x_c [BL, T, E] f32 -> [EK, 128, TP*BL] bf16 (optionally reversed)."""
    if rev:
        x_c = x_c[:, ::-1, :]
    xt = np.zeros((EK, 128, TP * BL), np.float32)
    # xt[e, p, t*BL + b] = x_c[b, t, 128e + p]
    xt[:, :, :T * BL] = np.ascontiguousarray(
        x_c.transpose(2, 1, 0)).reshape(EK, 128, T * BL)
    return xt.astype(BF)


def _prep_weights(W_ih, W_hh, b_ih, b_hh):
    Wi = np.array(W_ih, np.float32).copy()
    Wh = np.array(W_hh, np.float32).copy()
    Wi[H:2 * H] *= -1.0
    Wh[H:2 * H] *= -1.0
    # whh[k] = Whh'[:, 128k:128k+128].T  -> [128, G]
    whhT = np.ascontiguousarray(Wh.T).reshape(HK, 128, G).astype(BF)
    wihT = np.ascontiguousarray(Wi.T).reshape(EK, 128, G).astype(BF)
    bi = np.asarray(b_ih, np.float32)
    bh = np.asarray(b_hh, np.float32)
    vb = np.concatenate([
        bi[0:H] + bh[0:H],
        -(bi[H:2 * H] + bh[H:2 * H]),
        bi[2 * H:3 * H]])
    biasx = vb.reshape(NG, 128).astype(BF)
    bhn_ = bh[2 * H:3 * H].reshape(HK, 128).astype(BF)
    return wihT, whhT, biasx, bhn_


def prepare_in_maps(inputs, emb, W_ih_f, W_hh_f, b_ih_f, b_hh_f,
                    W_ih_b, W_hh_b, b_ih_b, b_hh_b, fc_w, fc_b):
    ids = np.asarray(inputs)
    emb = np.asarray(emb, np.float32)
    x = emb[ids]  # [B, T, E]

    indx_ = np.zeros((NG, NG * 128), np.float32)
    for k in range(NG):
        indx_[k, k * 128:(k + 1) * 128] = 1.0
    indh_ = np.zeros((HK, HK * BL), np.float32)
    for k in range(HK):
        indh_[k, k * BL:(k + 1) * BL] = 1.0
    fc = np.asarray(fc_w, np.float32)[0]
    ones = np.ones((1, BL), np.float32)

    per_dir = {}
    for d, (Wi, Wh, bi, bh) in (
            ("f", (W_ih_f, W_hh_f, b_ih_f, b_hh_f)),
            ("b", (W_ih_b, W_hh_b, b_ih_b, b_hh_b))):
        wihT, whhT, biasx, bhn_ = _prep_weights(Wi, Wh, bi, bh)
        off = 0 if d == "f" else H
        fcw_ = fc[off:off + H].reshape(HK, 128).T.copy().astype(BF)  # [128, HK]
        per_dir[d] = dict(whh=whhT, wih=wihT, biasx=biasx, bhn=bhn_,
                          fcw=fcw_, indx=indx_.astype(BF),
                          indh=indh_.astype(BF), ones=ones.astype(BF))

    in_maps = []
    for c in range(NC):
        d = "f" if c < 4 else "b"
        sh = c % 4
        x_c = x[sh * BL:(sh + 1) * BL]
        in_maps.append(dict(per_dir[d], xT=_prep_xT(x_c, d == "b")))
    return in_maps


def kernel(**inputs):
    in_maps = prepare_in_maps(**inputs)
    nc = _get_nc()
    res = run_bass_kernel_spmd(nc, in_maps, core_ids=list(range(NC)))
    fcb = np.float32(np.asarray(inputs["fc_b"], np.float32).reshape(-1)[0])
    out = np.empty((B, 1), np.float32)
    for sh in range(4):
        yf = np.asarray(res.results[sh]["y"], np.float32).reshape(BL)
        yb = np.asarray(res.results[4 + sh]["y"], np.float32).reshape(BL)
        out[sh * BL:(sh + 1) * BL, 0] = 1.0 / (1.0 + np.exp(-(yf + yb + fcb)))
    return out
